# revision 12
# baseline (speedup 1.0000x reference)
"""Cosine-similarity loss kernel for Trainium2 (8 NeuronCores, SPMD).

loss = -sum_n dot(s_n, im_n) / (||s_n|| * ||im_n||)   for s, im in R^{65536 x 512}

Strategy (memory-bound, ~390 GB/s HBM per core):
  - Shard the 65536 rows across 8 cores (8192 rows each, 32 MB/core).
  - Host-side repack: s and im rows are interleaved into ONE DRAM tensor
    laid out so each DMA tile is a single dma_start whose per-partition
    lines are fully contiguous (seg*4KB): tile t, partition p, slice j
    holds [s_row | im_row] (1024 f32) of row c*128+p, c = c0+j. Halves
    the dispatch + semaphore count vs two tensors and gives the SDMA
    engines ideal descriptors. Host repack cost doesn't count.
  - Per 128-row slice (64 slices/core), three fused one-pass reductions
    split across DVE and ACT at their measured rates (DVE 733ns/op,
    ACT 1000ns/op, DMA 1317ns/slice):
      dot = sum_d s*im  -> DVE scalar_tensor_tensor, accum_out
      ss  = sum_d s*s   -> ACT activation(Square, accum_out)
      ii  = sum_d im*im -> ~1/3 on ACT, rest on DVE; the last slices keep
            ii on DVE so the post-stream tail is DVE(dot+ii) || ACT(ss).
  - First tile's DMA is dispatched from the ACT HWDGE queue (its preamble
    retires earlier than Sync's), everything else from Sync.
  - Single ACT table set: warmup Abs_reciprocal_sqrt pins set 15 (which
    also contains Square) at startup -> no mid-kernel table switches.
  - Tail: loss_p[128,1] = -sum_c dot_c * abs_rsqrt(ss_c*ii_c); PE matmul
    against a ones vector folds the 128 partials into one PSUM scalar, so
    the output DMA is a single 4B descriptor.
  - Host sums the 8 per-core scalars -> f32.
"""

import numpy as np
from contextlib import ExitStack

import concourse.bacc as bacc
import concourse.bass as bass
import concourse.mybir as mybir
import concourse.tile as tile
from concourse.bass_utils import run_bass_kernel_spmd

N, D = 65536, 512
N_CORES = 8
ROWS = N // N_CORES          # 8192 rows per core
P = 128                      # SBUF partitions
SLICES = ROWS // P           # 64
F32 = mybir.dt.float32

# slices per DMA tile. Small first tiles start compute early; small last
# tiles shrink the post-DMA tail; large middle tiles keep dispatch and
# semaphore count low.
SEG_SCHEDULE = (1, 1, 2, 2, 2) + (4,) * 13 + (1, 1, 1, 1)
assert sum(SEG_SCHEDULE) == SLICES


def _emit_tail_part(nc, t, lo, hi):
    """loss_p[:, part] = -sum_{c in [lo,hi)} dot_c * abs_rsqrt(ss_c*ii_c)."""
    mult = mybir.AluOpType.mult
    part = 0 if lo == 0 else 1
    nc.vector.tensor_tensor(
        out=t["prod"][:, lo:hi], in0=t["ss_all"][:, lo:hi],
        in1=t["ii_all"][:, lo:hi], op=mult,
    )
    nc.scalar.activation(
        t["rt"][:, lo:hi], t["prod"][:, lo:hi],
        mybir.ActivationFunctionType.Abs_reciprocal_sqrt,
    )
    nc.vector.scalar_tensor_tensor(
        out=t["fin_scr"][:, lo:hi], in0=t["dot_all"][:, lo:hi], scalar=-1.0,
        in1=t["rt"][:, lo:hi], op0=mult, op1=mult,
        accum_out=t["loss_p"][:, part : part + 1],
    )


def _build(
    seg_schedule=SEG_SCHEDULE,
    bufs=8,
    split_c=56,      # tail part boundary: cols [0,split_c) reduced early
    # which slices' ii goes to ACT (~1/3), none in the last 4 so the
    # post-stream tail is DVE(dot+ii) || ACT(ss), not serialized on ACT
    ii_on_act=lambda c: ((c % 8) in (2, 5, 7) and c < 60) or c == 60,
    first_on_scalar=True,
):
    slices = sum(seg_schedule)

    nc = bacc.Bacc(
        "TRN2", target_bir_lowering=False, debug=False, num_devices=N_CORES
    )
    # interleaved input: tile t at row offset R_t, partition p, slice j
    # -> DRAM row R_t + p*seg + j = [s_row | im_row] of shard row c*128+p
    x_d = nc.dram_tensor("x", [ROWS, 2 * D], F32, kind="ExternalInput").ap()
    out_d = nc.dram_tensor("out", [2, 1], F32, kind="ExternalOutput").ap()

    mult = mybir.AluOpType.mult
    add = mybir.AluOpType.add

    with tile.TileContext(nc) as tc, ExitStack() as ctx:
        xpool = ctx.enter_context(tc.tile_pool(name="xpool", bufs=bufs))
        stats = ctx.enter_context(tc.tile_pool(name="stats", bufs=1))
        ppool = ctx.enter_context(
            tc.tile_pool(name="ppool", bufs=1, space="PSUM")
        )

        dot_all = stats.tile([P, slices], F32)
        ss_all = ppool.tile([P, slices], F32)
        ii_all = stats.tile([P, slices], F32)
        dve_scr = stats.tile([P, D], F32)
        act_scr = stats.tile([P, D], F32)
        prod = stats.tile([P, slices], F32)
        rt = stats.tile([P, slices], F32)
        fin_scr = stats.tile([P, slices], F32)
        loss_p = stats.tile([P, 2], F32)
        stats_tiles = dict(
            ii_all=ii_all, ss_all=ss_all, dot_all=dot_all,
            prod=prod, rt=rt, fin_scr=fin_scr, loss_p=loss_p,
        )

        # framework-preset [128,1] ones: ACT warmup input + final PE reduce
        ones = nc.const_aps.aps[(mybir.dt.float32, 1.0)]

        # Pin the ACT table to set 15 (abs_reciprocal_sqrt + square) before
        # the first Square, so the whole kernel needs exactly one table
        # load, issued during DMA warmup instead of at the tail.
        nc.scalar.activation(
            act_scr[:, 0:1], ones,
            mybir.ActivationFunctionType.Abs_reciprocal_sqrt,
        )

        c = 0
        r0 = 0
        for ti, seg in enumerate(seg_schedule):
            nrows = seg * P
            j2 = min(seg, 2)
            g = seg // j2
            x_seg = x_d[r0 : r0 + nrows, :].rearrange(
                "(g p j) f -> p g j f", g=g, p=P, j=j2
            )
            r0 += nrows
            xt = xpool.tile([P, g, j2, 2 * D], F32, name="xt", tag="xt")
            eng = nc.scalar if (first_on_scalar and ti == 0) else nc.sync
            eng.dma_start(xt[:], x_seg)
            for j in range(seg):
                st = xt[:, j // j2, j % j2, 0:D]
                it = xt[:, j // j2, j % j2, D : 2 * D]
                nc.vector.scalar_tensor_tensor(
                    out=dve_scr[:], in0=st, scalar=1.0, in1=it,
                    op0=mult, op1=mult,
                    accum_out=dot_all[:, c : c + 1],
                )
                nc.scalar.activation(
                    out=act_scr[:], in_=st,
                    func=mybir.ActivationFunctionType.Square,
                    accum_out=ss_all[:, c : c + 1],
                )
                if ii_on_act(c):
                    nc.scalar.activation(
                        out=act_scr[:], in_=it,
                        func=mybir.ActivationFunctionType.Square,
                        accum_out=ii_all[:, c : c + 1],
                    )
                else:
                    nc.vector.scalar_tensor_tensor(
                        out=dve_scr[:], in0=it, scalar=1.0, in1=it,
                        op0=mult, op1=mult,
                        accum_out=ii_all[:, c : c + 1],
                    )
                c += 1
                if c == split_c:
                    _emit_tail_part(nc, stats_tiles, 0, split_c)

        _emit_tail_part(nc, stats_tiles, split_c, slices)
        ps = ppool.tile([2, 1], F32)
        nc.tensor.matmul(ps[:], loss_p[:, 0:2], ones, start=True, stop=True)
        loss_s = stats.tile([2, 1], F32)
        nc.vector.tensor_copy(out=loss_s[:], in_=ps[:])
        nc.sync.dma_start(out_d, loss_s[:])

    nc.compile()
    return nc


_compiled = None


def _get_nc():
    global _compiled
    if _compiled is None:
        _compiled = _build()
    return _compiled


def _pack_core(s_shard, im_shard, seg_schedule=SEG_SCHEDULE):
    """Permute one core's rows into the DMA-friendly interleaved layout.

    Output row R_t + p*seg + j (tile t at row offset R_t) holds
    [s[c*128+p] | im[c*128+p]] with c = c0_t + j.
    """
    x = np.empty((ROWS, 2 * D), dtype=np.float32)
    r0 = 0
    c0 = 0
    for seg in seg_schedule:
        j2 = min(seg, 2)
        g = seg // j2
        sv = s_shard[c0 * P : (c0 + seg) * P].reshape(g, j2, P, D)
        iv = im_shard[c0 * P : (c0 + seg) * P].reshape(g, j2, P, D)
        blk = x[r0 : r0 + seg * P].reshape(g, P, j2, 2 * D)
        blk[:, :, :, 0:D] = sv.transpose(0, 2, 1, 3)
        blk[:, :, :, D : 2 * D] = iv.transpose(0, 2, 1, 3)
        r0 += seg * P
        c0 += seg
    return x


def _run(s, im, nc=None, **kw):
    """Pack, shard, run on 8 cores, return BassKernelResults."""
    s = np.ascontiguousarray(np.asarray(s, dtype=np.float32))
    im = np.ascontiguousarray(np.asarray(im, dtype=np.float32))
    assert s.shape == (N, D) and im.shape == (N, D)
    if nc is None:
        nc = _get_nc()
    in_maps = [
        {"x": _pack_core(s[c * ROWS : (c + 1) * ROWS],
                         im[c * ROWS : (c + 1) * ROWS])}
        for c in range(N_CORES)
    ]
    bkr = run_bass_kernel_spmd(nc, in_maps, core_ids=list(range(N_CORES)), **kw)
    return bkr


def kernel(s, im, temp=None, **_):
    bkr = _run(s, im)
    total = np.float64(0.0)
    for r in bkr.results:
        total += r["out"].astype(np.float64).sum()
    return np.float32(total)


# revision 13
# speedup vs baseline: 1.0034x; 1.0034x over previous
"""Cosine-similarity loss kernel for Trainium2 (8 NeuronCores, SPMD).

loss = -sum_n dot(s_n, im_n) / (||s_n|| * ||im_n||)   for s, im in R^{65536 x 512}

Strategy (memory-bound, ~390 GB/s HBM per core):
  - Shard the 65536 rows across 8 cores (8192 rows each, 32 MB/core).
  - Host-side repack: s and im rows are interleaved into ONE DRAM tensor
    laid out so each DMA tile is a single dma_start whose per-partition
    lines are fully contiguous (seg*4KB): tile t, partition p, slice j
    holds [s_row | im_row] (1024 f32) of row c*128+p, c = c0+j. Halves
    the dispatch + semaphore count vs two tensors and gives the SDMA
    engines ideal descriptors. Host repack cost doesn't count.
  - Per 128-row slice (64 slices/core), three fused one-pass reductions
    split across DVE and ACT at their measured rates (DVE 733ns/op,
    ACT 1000ns/op, DMA 1317ns/slice):
      dot = sum_d s*im  -> DVE scalar_tensor_tensor, accum_out
      ss  = sum_d s*s   -> ACT activation(Square, accum_out)
      ii  = sum_d im*im -> ~1/3 on ACT, rest on DVE; the last slices keep
            ii on DVE so the post-stream tail is DVE(dot+ii) || ACT(ss).
  - First tile's DMA is dispatched from the ACT HWDGE queue (its preamble
    retires earlier than Sync's), everything else from Sync.
  - Single ACT table set: warmup Abs_reciprocal_sqrt pins set 15 (which
    also contains Square) at startup -> no mid-kernel table switches.
  - Tail: loss_p[128,1] = -sum_c dot_c * abs_rsqrt(ss_c*ii_c); PE matmul
    against a ones vector folds the 128 partials into one PSUM scalar, so
    the output DMA is a single 4B descriptor.
  - Host sums the 8 per-core scalars -> f32.
"""

import numpy as np
from contextlib import ExitStack

import concourse.bacc as bacc
import concourse.bass as bass
import concourse.mybir as mybir
import concourse.tile as tile
from concourse.bass_utils import run_bass_kernel_spmd

N, D = 65536, 512
N_CORES = 8
ROWS = N // N_CORES          # 8192 rows per core
P = 128                      # SBUF partitions
SLICES = ROWS // P           # 64
F32 = mybir.dt.float32

# slices per DMA tile. Small first tiles start compute early; small last
# tiles shrink the post-DMA tail; large middle tiles keep dispatch and
# semaphore count low.
SEG_SCHEDULE = (1, 1, 2) + (4,) * 14 + (1, 1, 1, 1)
assert sum(SEG_SCHEDULE) == SLICES


def _emit_tail_part(nc, t, lo, hi):
    """loss_p[:, part] = -sum_{c in [lo,hi)} dot_c * abs_rsqrt(ss_c*ii_c)."""
    mult = mybir.AluOpType.mult
    part = 0 if lo == 0 else 1
    nc.vector.tensor_tensor(
        out=t["prod"][:, lo:hi], in0=t["ss_all"][:, lo:hi],
        in1=t["ii_all"][:, lo:hi], op=mult,
    )
    nc.scalar.activation(
        t["rt"][:, lo:hi], t["prod"][:, lo:hi],
        mybir.ActivationFunctionType.Abs_reciprocal_sqrt,
    )
    nc.vector.scalar_tensor_tensor(
        out=t["fin_scr"][:, lo:hi], in0=t["dot_all"][:, lo:hi], scalar=-1.0,
        in1=t["rt"][:, lo:hi], op0=mult, op1=mult,
        accum_out=t["loss_p"][:, part : part + 1],
    )


def _build(
    seg_schedule=SEG_SCHEDULE,
    bufs=8,
    split_c=56,      # tail part boundary: cols [0,split_c) reduced early
    # which slices' ii goes to ACT (~1/3), none in the last 4 so the
    # post-stream tail is DVE(dot+ii) || ACT(ss), not serialized on ACT
    ii_on_act=lambda c: ((c % 8) in (2, 5, 7) and c < 60) or c == 60,
    first_on_scalar=True,
):
    slices = sum(seg_schedule)

    nc = bacc.Bacc(
        "TRN2", target_bir_lowering=False, debug=False, num_devices=N_CORES
    )
    # interleaved input: tile t at row offset R_t, partition p, slice j
    # -> DRAM row R_t + p*seg + j = [s_row | im_row] of shard row c*128+p
    x_d = nc.dram_tensor("x", [ROWS, 2 * D], F32, kind="ExternalInput").ap()
    out_d = nc.dram_tensor("out", [2, 1], F32, kind="ExternalOutput").ap()

    mult = mybir.AluOpType.mult
    add = mybir.AluOpType.add

    with tile.TileContext(nc) as tc, ExitStack() as ctx:
        xpool = ctx.enter_context(tc.tile_pool(name="xpool", bufs=bufs))
        stats = ctx.enter_context(tc.tile_pool(name="stats", bufs=1))
        ppool = ctx.enter_context(
            tc.tile_pool(name="ppool", bufs=1, space="PSUM")
        )

        dot_all = stats.tile([P, slices], F32)
        ss_all = ppool.tile([P, slices], F32)
        ii_all = stats.tile([P, slices], F32)
        dve_scr = stats.tile([P, D], F32)
        act_scr = stats.tile([P, D], F32)
        prod = stats.tile([P, slices], F32)
        rt = stats.tile([P, slices], F32)
        fin_scr = stats.tile([P, slices], F32)
        loss_p = stats.tile([P, 2], F32)
        stats_tiles = dict(
            ii_all=ii_all, ss_all=ss_all, dot_all=dot_all,
            prod=prod, rt=rt, fin_scr=fin_scr, loss_p=loss_p,
        )

        # framework-preset [128,1] ones: ACT warmup input + final PE reduce
        ones = nc.const_aps.aps[(mybir.dt.float32, 1.0)]

        # Pin the ACT table to set 15 (abs_reciprocal_sqrt + square) before
        # the first Square, so the whole kernel needs exactly one table
        # load, issued during DMA warmup instead of at the tail.
        nc.scalar.activation(
            act_scr[:, 0:1], ones,
            mybir.ActivationFunctionType.Abs_reciprocal_sqrt,
        )

        c = 0
        r0 = 0
        for ti, seg in enumerate(seg_schedule):
            nrows = seg * P
            j2 = min(seg, 2)
            g = seg // j2
            x_seg = x_d[r0 : r0 + nrows, :].rearrange(
                "(g p j) f -> p g j f", g=g, p=P, j=j2
            )
            r0 += nrows
            xt = xpool.tile([P, g, j2, 2 * D], F32, name="xt", tag="xt")
            eng = nc.scalar if (first_on_scalar and ti == 0) else nc.sync
            eng.dma_start(xt[:], x_seg)
            for j in range(seg):
                st = xt[:, j // j2, j % j2, 0:D]
                it = xt[:, j // j2, j % j2, D : 2 * D]
                nc.vector.scalar_tensor_tensor(
                    out=dve_scr[:], in0=st, scalar=1.0, in1=it,
                    op0=mult, op1=mult,
                    accum_out=dot_all[:, c : c + 1],
                )
                nc.scalar.activation(
                    out=act_scr[:], in_=st,
                    func=mybir.ActivationFunctionType.Square,
                    accum_out=ss_all[:, c : c + 1],
                )
                if ii_on_act(c):
                    nc.scalar.activation(
                        out=act_scr[:], in_=it,
                        func=mybir.ActivationFunctionType.Square,
                        accum_out=ii_all[:, c : c + 1],
                    )
                else:
                    nc.vector.scalar_tensor_tensor(
                        out=dve_scr[:], in0=it, scalar=1.0, in1=it,
                        op0=mult, op1=mult,
                        accum_out=ii_all[:, c : c + 1],
                    )
                c += 1
                if c == split_c:
                    _emit_tail_part(nc, stats_tiles, 0, split_c)

        _emit_tail_part(nc, stats_tiles, split_c, slices)
        ps = ppool.tile([2, 1], F32)
        nc.tensor.matmul(ps[:], loss_p[:, 0:2], ones, start=True, stop=True)
        loss_s = stats.tile([2, 1], F32)
        nc.vector.tensor_copy(out=loss_s[:], in_=ps[:])
        nc.sync.dma_start(out_d, loss_s[:])

    nc.compile()
    return nc


_compiled = None


def _get_nc():
    global _compiled
    if _compiled is None:
        _compiled = _build()
    return _compiled


def _pack_core(s_shard, im_shard, seg_schedule=SEG_SCHEDULE):
    """Permute one core's rows into the DMA-friendly interleaved layout.

    Output row R_t + p*seg + j (tile t at row offset R_t) holds
    [s[c*128+p] | im[c*128+p]] with c = c0_t + j.
    """
    x = np.empty((ROWS, 2 * D), dtype=np.float32)
    r0 = 0
    c0 = 0
    for seg in seg_schedule:
        j2 = min(seg, 2)
        g = seg // j2
        sv = s_shard[c0 * P : (c0 + seg) * P].reshape(g, j2, P, D)
        iv = im_shard[c0 * P : (c0 + seg) * P].reshape(g, j2, P, D)
        blk = x[r0 : r0 + seg * P].reshape(g, P, j2, 2 * D)
        blk[:, :, :, 0:D] = sv.transpose(0, 2, 1, 3)
        blk[:, :, :, D : 2 * D] = iv.transpose(0, 2, 1, 3)
        r0 += seg * P
        c0 += seg
    return x


def _run(s, im, nc=None, **kw):
    """Pack, shard, run on 8 cores, return BassKernelResults."""
    s = np.ascontiguousarray(np.asarray(s, dtype=np.float32))
    im = np.ascontiguousarray(np.asarray(im, dtype=np.float32))
    assert s.shape == (N, D) and im.shape == (N, D)
    if nc is None:
        nc = _get_nc()
    in_maps = [
        {"x": _pack_core(s[c * ROWS : (c + 1) * ROWS],
                         im[c * ROWS : (c + 1) * ROWS])}
        for c in range(N_CORES)
    ]
    bkr = run_bass_kernel_spmd(nc, in_maps, core_ids=list(range(N_CORES)), **kw)
    return bkr


def kernel(s, im, temp=None, **_):
    bkr = _run(s, im)
    total = np.float64(0.0)
    for r in bkr.results:
        total += r["out"].astype(np.float64).sum()
    return np.float32(total)


# revision 14
# speedup vs baseline: 1.0165x; 1.0130x over previous
"""Cosine-similarity loss kernel for Trainium2 (8 NeuronCores, SPMD).

loss = -sum_n dot(s_n, im_n) / (||s_n|| * ||im_n||)   for s, im in R^{65536 x 512}

Strategy (memory-bound, ~390 GB/s HBM per core):
  - Shard the 65536 rows across 8 cores (8192 rows each, 32 MB/core).
  - Host-side repack: s and im rows are interleaved into ONE DRAM tensor
    laid out so each DMA tile is a single dma_start whose per-partition
    lines are fully contiguous (seg*4KB): tile t, partition p, slice j
    holds [s_row | im_row] (1024 f32) of row c*128+p, c = c0+j. Halves
    the dispatch + semaphore count vs two tensors and gives the SDMA
    engines ideal descriptors. Host repack cost doesn't count.
  - Per 128-row slice (64 slices/core), three fused one-pass reductions
    split across DVE and ACT at their measured rates (DVE 733ns/op,
    ACT 1000ns/op, DMA 1317ns/slice):
      dot = sum_d s*im  -> DVE scalar_tensor_tensor, accum_out
      ss  = sum_d s*s   -> ACT activation(Square, accum_out)
      ii  = sum_d im*im -> ~1/3 on ACT, rest on DVE; the last slices keep
            ii on DVE so the post-stream tail is DVE(dot+ii) || ACT(ss).
  - First tile's DMA is dispatched from the ACT HWDGE queue (its preamble
    retires earlier than Sync's), everything else from Sync.
  - Single ACT table set: warmup Abs_reciprocal_sqrt pins set 15 (which
    also contains Square) at startup -> no mid-kernel table switches.
  - Tail: loss_p[128,1] = -sum_c dot_c * abs_rsqrt(ss_c*ii_c); PE matmul
    against a ones vector folds the 128 partials into one PSUM scalar, so
    the output DMA is a single 4B descriptor.
  - Host sums the 8 per-core scalars -> f32.
"""

import numpy as np
from contextlib import ExitStack

import concourse.bacc as bacc
import concourse.bass as bass
import concourse.mybir as mybir
import concourse.tile as tile
from concourse.bass_utils import run_bass_kernel_spmd

N, D = 65536, 512
N_CORES = 8
ROWS = N // N_CORES          # 8192 rows per core
P = 128                      # SBUF partitions
SLICES = ROWS // P           # 64
F32 = mybir.dt.float32

# slices per DMA tile. Small first tiles start compute early; small last
# tiles shrink the post-DMA tail; large middle tiles keep dispatch and
# semaphore count low.
SEG_SCHEDULE = (1, 1, 2) + (4,) * 14 + (1, 1, 1, 1)
assert sum(SEG_SCHEDULE) == SLICES


def _emit_tail_part(nc, t, lo, hi):
    """loss_p[:, part] = -sum_{c in [lo,hi)} dot_c * abs_rsqrt(ss_c*ii_c)."""
    mult = mybir.AluOpType.mult
    part = 0 if lo == 0 else 1
    nc.vector.tensor_tensor(
        out=t["prod"][:, lo:hi], in0=t["ss_all"][:, lo:hi],
        in1=t["ii_all"][:, lo:hi], op=mult,
    )
    nc.scalar.activation(
        t["rt"][:, lo:hi], t["prod"][:, lo:hi],
        mybir.ActivationFunctionType.Abs_reciprocal_sqrt,
    )
    nc.vector.scalar_tensor_tensor(
        out=t["fin_scr"][:, lo:hi], in0=t["dot_all"][:, lo:hi], scalar=-1.0,
        in1=t["rt"][:, lo:hi], op0=mult, op1=mult,
        accum_out=t["loss_p"][:, part : part + 1],
    )


def _build(
    seg_schedule=SEG_SCHEDULE,
    bufs=8,
    split_c=56,      # tail part boundary: cols [0,split_c) reduced early
    # which slices' ii goes to ACT (~1/3), none in the last 4 so the
    # post-stream tail is DVE(dot+ii) || ACT(ss), not serialized on ACT
    ii_on_act=lambda c: ((c % 8) in (2, 5, 7) and c < 60) or c == 60,
    first_on_scalar=True,
):
    slices = sum(seg_schedule)

    nc = bacc.Bacc(
        "TRN2", target_bir_lowering=False, debug=False, num_devices=N_CORES
    )
    # interleaved input: tile t at row offset R_t, partition p, slice j
    # -> DRAM row R_t + p*seg + j = [s_row | im_row] of shard row c*128+p
    x_d = nc.dram_tensor("x", [ROWS, 2 * D], F32, kind="ExternalInput").ap()
    out_d = nc.dram_tensor("out", [2, 1], F32, kind="ExternalOutput").ap()

    mult = mybir.AluOpType.mult
    add = mybir.AluOpType.add

    with tile.TileContext(nc) as tc, ExitStack() as ctx:
        xpool = ctx.enter_context(tc.tile_pool(name="xpool", bufs=bufs))
        stats = ctx.enter_context(tc.tile_pool(name="stats", bufs=1))
        ppool = ctx.enter_context(
            tc.tile_pool(name="ppool", bufs=1, space="PSUM")
        )

        dot_all = stats.tile([P, slices], F32)
        ss_all = ppool.tile([P, slices], F32)
        ii_all = stats.tile([P, slices], F32)
        dve_scr = stats.tile([P, D], F32)
        act_scr = stats.tile([P, D], F32)
        prod = stats.tile([P, slices], F32)
        rt = stats.tile([P, slices], F32)
        fin_scr = stats.tile([P, slices], F32)
        loss_p = stats.tile([P, 2], F32)
        stats_tiles = dict(
            ii_all=ii_all, ss_all=ss_all, dot_all=dot_all,
            prod=prod, rt=rt, fin_scr=fin_scr, loss_p=loss_p,
        )

        # framework-preset [128,1] ones: ACT warmup input + final PE reduce
        ones = nc.const_aps.aps[(mybir.dt.float32, 1.0)]

        # Pin the ACT table to set 15 (abs_reciprocal_sqrt + square) before
        # the first Square, so the whole kernel needs exactly one table
        # load, issued during DMA warmup instead of at the tail.
        nc.scalar.activation(
            act_scr[:, 0:1], ones,
            mybir.ActivationFunctionType.Abs_reciprocal_sqrt,
        )

        c = 0
        r0 = 0
        for ti, seg in enumerate(seg_schedule):
            nrows = seg * P
            j2 = min(seg, 2)
            g = seg // j2
            x_seg = x_d[r0 : r0 + nrows, :].rearrange(
                "(g p j) f -> p g j f", g=g, p=P, j=j2
            )
            r0 += nrows
            xt = xpool.tile([P, g, j2, 2 * D], F32, name="xt", tag="xt")
            eng = nc.scalar if (first_on_scalar and ti == 0) else nc.sync
            # one dma_start per g-half: cheap 3-D descriptors and finer
            # completion granularity, so compute tracks arrivals closer
            for gi in range(g):
                eng.dma_start(xt[:, gi], x_seg[:, gi])
            for j in range(seg):
                st = xt[:, j // j2, j % j2, 0:D]
                it = xt[:, j // j2, j % j2, D : 2 * D]
                nc.vector.scalar_tensor_tensor(
                    out=dve_scr[:], in0=st, scalar=1.0, in1=it,
                    op0=mult, op1=mult,
                    accum_out=dot_all[:, c : c + 1],
                )
                nc.scalar.activation(
                    out=act_scr[:], in_=st,
                    func=mybir.ActivationFunctionType.Square,
                    accum_out=ss_all[:, c : c + 1],
                )
                if ii_on_act(c):
                    nc.scalar.activation(
                        out=act_scr[:], in_=it,
                        func=mybir.ActivationFunctionType.Square,
                        accum_out=ii_all[:, c : c + 1],
                    )
                else:
                    nc.vector.scalar_tensor_tensor(
                        out=dve_scr[:], in0=it, scalar=1.0, in1=it,
                        op0=mult, op1=mult,
                        accum_out=ii_all[:, c : c + 1],
                    )
                c += 1
                if c == split_c:
                    _emit_tail_part(nc, stats_tiles, 0, split_c)

        _emit_tail_part(nc, stats_tiles, split_c, slices)
        ps = ppool.tile([2, 1], F32)
        nc.tensor.matmul(ps[:], loss_p[:, 0:2], ones, start=True, stop=True)
        loss_s = stats.tile([2, 1], F32)
        nc.vector.tensor_copy(out=loss_s[:], in_=ps[:])
        nc.sync.dma_start(out_d, loss_s[:])

    nc.compile()
    return nc


_compiled = None


def _get_nc():
    global _compiled
    if _compiled is None:
        _compiled = _build()
    return _compiled


def _pack_core(s_shard, im_shard, seg_schedule=SEG_SCHEDULE):
    """Permute one core's rows into the DMA-friendly interleaved layout.

    Output row R_t + p*seg + j (tile t at row offset R_t) holds
    [s[c*128+p] | im[c*128+p]] with c = c0_t + j.
    """
    x = np.empty((ROWS, 2 * D), dtype=np.float32)
    r0 = 0
    c0 = 0
    for seg in seg_schedule:
        j2 = min(seg, 2)
        g = seg // j2
        sv = s_shard[c0 * P : (c0 + seg) * P].reshape(g, j2, P, D)
        iv = im_shard[c0 * P : (c0 + seg) * P].reshape(g, j2, P, D)
        blk = x[r0 : r0 + seg * P].reshape(g, P, j2, 2 * D)
        blk[:, :, :, 0:D] = sv.transpose(0, 2, 1, 3)
        blk[:, :, :, D : 2 * D] = iv.transpose(0, 2, 1, 3)
        r0 += seg * P
        c0 += seg
    return x


def _run(s, im, nc=None, **kw):
    """Pack, shard, run on 8 cores, return BassKernelResults."""
    s = np.ascontiguousarray(np.asarray(s, dtype=np.float32))
    im = np.ascontiguousarray(np.asarray(im, dtype=np.float32))
    assert s.shape == (N, D) and im.shape == (N, D)
    if nc is None:
        nc = _get_nc()
    in_maps = [
        {"x": _pack_core(s[c * ROWS : (c + 1) * ROWS],
                         im[c * ROWS : (c + 1) * ROWS])}
        for c in range(N_CORES)
    ]
    bkr = run_bass_kernel_spmd(nc, in_maps, core_ids=list(range(N_CORES)), **kw)
    return bkr


def kernel(s, im, temp=None, **_):
    bkr = _run(s, im)
    total = np.float64(0.0)
    for r in bkr.results:
        total += r["out"].astype(np.float64).sum()
    return np.float32(total)
